# revision 15
# baseline (speedup 1.0000x reference)
"""CTAttention Trainium2 kernel.

Full-input contract: kernel(**inputs) takes the unsharded numpy inputs and
returns the full [total, C] output. Internally: data-parallel over the batch
axis B=8 across 8 NeuronCores (attention is independent per batch element);
qkv/proj weights replicated; ragged scatter/gather bookkeeping on the host.

Per-core dataflow (batch b, dense 1024 windows, 8 heads, head_dim 32):
  X^T[256,1024] (bf16) -> Q^T/K^T bf16 (channel-on-partition) and V[kpos,ch]
  attention runs 4 heads (one group) at a time, software-pipelined:
    scores: S^T = per-head K=32 bf16 matmuls, 4-way row-packed on the PE
            array (one 32-row strip per head), into two [128,1024] psums
    exp:    one ScalarE Exp per psum tile with the key-padding mask as a
            per-partition bias (masked scores underflow to exactly 0, so no
            row-max pass is needed); output P^T in bf16
    PV:     bf16 matmuls col-tiled across the 4 PE column strips; an extra
            M=1 all-ones matmul per head accumulates the softmax denominator
            into a spare psum row of the pair accumulator
  normalization: denominators -> 1/x via ScalarE Ln then Exp(-x) (both live in
  the same ACT table set, so no table switches), broadcast to all 32-row
  strips with a single K=4 selector matmul, one DVE multiply per half.
  Output projection in bf16 from the strip-assembled O^T.

All matmul operands are bf16: on this hardware f32r matmuls execute in
fp32_mode=HIGH (4 passes, ~4x slower than bf16), so every f32r path was
converted. PSUM accumulation stays fp32.

Exact algebraic simplifications vs the reference:
  - K bias dropped (softmax is invariant to per-query constant shifts)
  - V bias folded into the proj bias (softmax weights sum to 1)
  - head-dim scale folded into the exp's input scale

Environment workarounds (this walrus build): at most one sem wait per
instruction (waits hoisted onto injected NOPs), fp32/fp32r matmuls require
dst partition base 0, no gpsimd extended instructions, no custom DVE ops.
"""

import sys

if "/opt/trn_rl_repo" not in sys.path:
    sys.path.insert(0, "/opt/trn_rl_repo")

import numpy as np

B = 8
C = 256
H = 8
HD = 32
MAXW = 1024
SCALE = HD ** -0.5
NEG_THRESH = -1e8  # mask values below this count as fully masked

_cached = {}


def _build_nc():
    import bass_rust
    import concourse.bass as bass
    import concourse.tile as tile
    import concourse.mybir as mybir
    from concourse.vector_clock import ScopedClock

    # ---- workaround: this walrus build accepts at most ONE sem wait per
    # instruction ("Too many sync wait commands" in setupSyncWait). Tile
    # attaches multi-sem waits freely. Split: hoist all but the last wait of
    # every committed instruction onto injected same-engine NOPs, and split
    # the final drain the same way.
    _ctr = [0]

    def _hoist_excess_waits(tc_self, inst, orig_add):
        si = inst.sync_info
        if si is not None:
            waits = list(si.on_wait or [])
            if len(waits) > 1:
                for w in waits[:-1]:
                    _ctr[0] += 1
                    nop = mybir.InstNoOp(name=f"waitsplit-{_ctr[0]}")
                    nop.engine = inst.engine
                    nop.sync_info = bass_rust.SyncInfo(on_wait=[w], on_update=[])
                    orig_add(tc_self, nop)
                si.on_wait = waits[-1:]
        orig_add(tc_self, inst)

    if not getattr(tile.TileContext, "_waitsplit_patched", False):
        _orig_add_instruction = tile.TileContext._add_instruction

        def _split_add_instruction(self, inst):
            _hoist_excess_waits(self, inst, _orig_add_instruction)

        tile.TileContext._add_instruction = _split_add_instruction

        def _patched_drain_and_barrier(self, tick_clock, wait_clock):
            nc = self.nc
            d0 = nc.sync.drain()
            wait_clock.add_sem_waits(
                d0.ins, ScopedClock({None: tick_clock.global_clock})
            )
            si = d0.ins.sync_info
            waits = list(si.on_wait) if si is not None else []
            if len(waits) > 1:
                si.on_wait = waits[0:1]
                for w in waits[1:]:
                    dk = nc.sync.drain()
                    dk.ins.sync_info = bass_rust.SyncInfo(on_wait=[w], on_update=[])
            nc.all_engine_barrier()
            assert self.sems is not None
            popped = nc._tile_sem_poison_stack.pop()
            assert popped is self._sem_poison
            nc.clear_and_free_semaphores(list(self.sems.allocated().values()))
            nc.all_engine_barrier()

        tile.TileContext._drain_and_barrier = _patched_drain_and_barrier
        tile.TileContext._waitsplit_patched = True

    dt = mybir.dt
    f32 = dt.float32
    bf16 = dt.bfloat16
    AF = mybir.ActivationFunctionType

    nc = bass.Bass(
        "TRN2",
        target_bir_lowering=False,
        debug=False,
        num_devices=1,
        enable_asserts=False,
    )

    xt_d = nc.dram_tensor("xt", [128, 2048], bf16, kind="ExternalInput").ap()
    qw_d = nc.dram_tensor("qw", [128, 1536], bf16, kind="ExternalInput").ap()
    qb_d = nc.dram_tensor("qb", [128, 2], f32, kind="ExternalInput").ap()
    pw_d = nc.dram_tensor("pw", [128, 512], bf16, kind="ExternalInput").ap()
    pb_d = nc.dram_tensor("pb", [128, 2], f32, kind="ExternalInput").ap()
    mask_d = nc.dram_tensor("mask", [128, 8], f32, kind="ExternalInput").ap()
    onesb_d = nc.dram_tensor("onesb", [128, 4], bf16, kind="ExternalInput").ap()
    sel_d = nc.dram_tensor("sel", [4, 128], bf16, kind="ExternalInput").ap()
    yt_d = nc.dram_tensor("yt", [128, 2048], f32, kind="ExternalOutput").ap()

    with tile.TileContext(nc) as tc:
        with (
            tc.tile_pool(name="const", bufs=1) as const_pool,
            tc.tile_pool(name="big", bufs=1) as big_pool,
            tc.tile_pool(name="pt", bufs=6) as pt_pool,
            tc.tile_pool(name="stage", bufs=4) as stage_pool,
            tc.tile_pool(name="norm", bufs=2) as norm_pool,
            tc.tile_pool(name="ps_s4a", bufs=1, space="PSUM") as ps_s4a,
            tc.tile_pool(name="ps_s4b", bufs=1, space="PSUM") as ps_s4b,
            tc.tile_pool(name="ps_ot", bufs=2, space="PSUM") as ps_ot,
        ):
            xt = const_pool.tile([128, 2048], bf16, tag="xt")
            qw = const_pool.tile([128, 1536], bf16, tag="qw")
            qb = const_pool.tile([128, 2], f32, tag="qb")
            pw = const_pool.tile([128, 512], bf16, tag="pw")
            pb = const_pool.tile([128, 2], f32, tag="pb")
            mask = const_pool.tile([128, 8], f32, tag="mask")
            onesb = const_pool.tile([128, 4], bf16, tag="onesb")
            sel = const_pool.tile([4, 128], bf16, tag="sel")

            # warm the Exp table immediately (independent of any input DMA)
            warm = const_pool.tile([4, 128], f32, tag="warm")
            nc.vector.memset(warm[:], 0.0)
            nc.scalar.activation(warm[:], warm[:], AF.Exp, scale=0.0)

            # input DMA, ordered by first use: the first matmuls (qk0/qk2,
            # chunk c=0) need qw m-strips 0 and 2 plus xt chunks 0 and 2.
            def qw_strips(m):
                return [(128 * m, 128 * (m + 1)), (768 + 128 * m, 896 + 128 * m)]

            nc.gpsimd.dma_start(mask[:], mask_d)
            nc.gpsimd.dma_start(sel[:], sel_d)
            for lo, hi in qw_strips(0):
                nc.scalar.dma_start(qw[:, lo:hi], qw_d[:, lo:hi])
            for lo, hi in qw_strips(2):
                nc.sync.dma_start(qw[:, lo:hi], qw_d[:, lo:hi])
            nc.sync.dma_start(xt[:, 0:512], xt_d[:, 0:512])
            nc.scalar.dma_start(xt[:, 1024:1536], xt_d[:, 1024:1536])
            nc.scalar.dma_start(xt[:, 512:1024], xt_d[:, 512:1024])
            nc.sync.dma_start(xt[:, 1536:2048], xt_d[:, 1536:2048])
            nc.sync.dma_start(qw[:, 512:768], qw_d[:, 512:768])  # V strip t0
            nc.scalar.dma_start(qw[:, 1280:1536], qw_d[:, 1280:1536])  # V strip t1
            for m in (1, 3):
                for k, (lo, hi) in enumerate(qw_strips(m)):
                    (nc.sync if k == 0 else nc.scalar).dma_start(
                        qw[:, lo:hi], qw_d[:, lo:hi]
                    )
            nc.gpsimd.dma_start(qb[:], qb_d)
            nc.gpsimd.dma_start(pw[:], pw_d)
            nc.gpsimd.dma_start(pb[:], pb_d)
            nc.gpsimd.dma_start(onesb[:], onesb_d)

            qt = big_pool.tile([128, 2048], bf16, tag="qt")
            kt = big_pool.tile([128, 2048], bf16, tag="kt")
            va = big_pool.tile([128, 8, 8, 32], bf16, tag="va")  # [part, kpos_blk, head, head_dim]
            otf = big_pool.tile([128, 2048], bf16, tag="otf")
            ytile = big_pool.tile([128, 2048], f32, tag="ytile")

            # ---------- qkv projections ----------
            # V first (PV needs all of it), then the Q/K tiles needed by the
            # first head group; the rest is emitted between the groups so the
            # PE fills ACT-bound gaps.
            def qk_tile(m, chunks=(0, 1), pool=None, ptag="ot"):
                pool = pool if pool is not None else ps_ot
                for c in chunks:
                    ps = pool.tile([128, 512], f32, tag=ptag, name=f"qk{m}{c}")
                    for t in range(2):
                        nc.tensor.matmul(
                            ps[:],
                            qw[:, 768 * t + 128 * m : 768 * t + 128 * (m + 1)],
                            xt[:, 1024 * t + 512 * c : 1024 * t + 512 * (c + 1)],
                            start=(t == 0),
                            stop=(t == 1),
                        )
                    if m < 2:
                        nc.vector.tensor_scalar_add(
                            qt[:, 1024 * m + 512 * c : 1024 * m + 512 * (c + 1)],
                            ps[:],
                            qb[:, m : m + 1],
                        )
                    else:
                        nc.vector.tensor_copy(
                            kt[:, 1024 * (m - 2) + 512 * c : 1024 * (m - 2) + 512 * (c + 1)],
                            ps[:],
                        )

            # V: out[kpos_block, cv] in bf16 (no bias; folded into proj bias)
            def v_tile(j, pool=None, ptag="ot"):
                pool = pool if pool is not None else ps_ot
                ps = pool.tile([128, 256], f32, tag=ptag, name=f"v{j}")
                for t in range(2):
                    nc.tensor.matmul(
                        ps[:],
                        xt[:, 1024 * t + 128 * j : 1024 * t + 128 * (j + 1)],
                        qw[:, 768 * t + 512 : 768 * t + 768],
                        start=(t == 0),
                        stop=(t == 1),
                    )
                nc.vector.tensor_copy(
                    va[:, j, :, :],
                    ps[:].rearrange("p (h d) -> p h d", d=32),
                )

            qk_tile(0, chunks=(0,))
            qk_tile(2, chunks=(0,))

            # ---------- attention: 4 heads (one group) at a time ----------
            # scores: 4-way row-packed bf16 matmuls, two [128,1024] psum tiles
            # per step (double-buffered so ScalarE exps run back-to-back), one
            # exp per tile -> bf16 P^T, then 8 bf16 PV/rowsum matmuls col-tiled
            # across the 4 PE column strips.
            # O^T strips: head hh at psum rows 32*hh of its pair psum
            # (pair 0 = hh 0,1 rows 0-63; pair 1 = hh 2,3 rows 64-127);
            # denominator rows: hh -> (64, 96, 0, 32).
            # The (grp, j, c) iterations run as ONE flat software pipeline so
            # group 1's first scores overlap group 0's last PV; remaining
            # qkv-projection tiles are spread across the early iterations to
            # fill the initial exp latencies without delaying the first exp.
            ovs = {}

            def emit_scores(grp, j, c):
                s4a = ps_s4a.tile([128, 1024], f32, tag="s4a", name=f"s4a{grp}{j}{c}")
                s4b = ps_s4b.tile([128, 1024], f32, tag="s4b", name=f"s4b{grp}{j}{c}")
                for hh in range(4):
                    s4 = s4a if hh < 2 else s4b
                    base = 32 * hh
                    nc.tensor.matmul(
                        s4[:, 512 * (hh % 2) : 512 * (hh % 2 + 1)],
                        kt[base : base + 32,
                           1024 * grp + 128 * j : 1024 * grp + 128 * (j + 1)],
                        qt[base : base + 32,
                           1024 * grp + 512 * c : 1024 * grp + 512 * (c + 1)],
                        start=True,
                        stop=True,
                        tile_position=(base, 0),
                    )
                pta = pt_pool.tile([128, 1024], bf16, tag="pt", name=f"pta{grp}{j}{c}")
                ptb = pt_pool.tile([128, 1024], bf16, tag="pt", name=f"ptb{grp}{j}{c}")
                nc.scalar.activation(
                    pta[:], s4a[:], AF.Exp, bias=mask[:, j : j + 1], scale=SCALE,
                )
                nc.scalar.activation(
                    ptb[:], s4b[:], AF.Exp, bias=mask[:, j : j + 1], scale=SCALE,
                )
                return pta, ptb

            def emit_pv(pta, ptb, grp, j, c):
                ov0, ov1 = ovs[grp]
                sj = (j == 0)
                ej = (j == 7)
                for hh in range(4):
                    h = 4 * grp + hh
                    ov = ov0 if hh < 2 else ov1
                    pt = pta if hh < 2 else ptb
                    vpos = 32 * hh
                    nc.tensor.matmul(
                        ov[vpos : vpos + 32, 512 * c : 512 * (c + 1)],
                        va[:, j, h, :],
                        pt[:, 512 * (hh % 2) : 512 * (hh % 2 + 1)],
                        start=sj,
                        stop=ej,
                        tile_position=(0, vpos),
                    )
                for hh in range(4):
                    ov = ov0 if hh < 2 else ov1
                    pt = pta if hh < 2 else ptb
                    spos = (64, 96, 0, 32)[hh]
                    nc.tensor.matmul(
                        ov[spos : spos + 1, 512 * c : 512 * (c + 1)],
                        onesb[:, 0:1],
                        pt[:, 512 * (hh % 2) : 512 * (hh % 2 + 1)],
                        start=sj,
                        stop=ej,
                        tile_position=(0, spos),
                    )

            saved = {}

            def emit_group_stage(grp):
                """Staging + reciprocal for a finished group (bc/mult deferred
                to the tail: the ps_ot slots are taken by the live ov
                accumulators until then). Group 1 (the tail-critical one)
                stages in c-halves so the reciprocal chain starts after half
                the copy work."""
                ov0, ov1 = ovs[grp]
                st0 = stage_pool.tile([128, 1024], f32, tag="st", name=f"st0_{grp}")
                st1 = stage_pool.tile([128, 1024], f32, tag="st", name=f"st1_{grp}")
                se4 = norm_pool.tile([4, 1024], f32, tag="se4", name=f"se4_{grp}")
                ln4 = norm_pool.tile([4, 1024], f32, tag="ln4", name=f"ln4_{grp}")
                rc4 = norm_pool.tile([4, 1024], bf16, tag="rc4", name=f"rc4_{grp}")
                if grp == 1:
                    for c in range(2):
                        cs = slice(512 * c, 512 * (c + 1))
                        nc.vector.tensor_copy(st0[:, cs], ov0[:, cs])
                        nc.scalar.copy(st1[:, cs], ov1[:, cs])
                        nc.sync.dma_start(se4[0:1, cs], st0[64:65, cs])
                        nc.scalar.dma_start(se4[1:2, cs], st0[96:97, cs])
                        nc.sync.dma_start(se4[2:3, cs], st1[0:1, cs])
                        nc.scalar.dma_start(se4[3:4, cs], st1[32:33, cs])
                        nc.scalar.activation(ln4[:, cs], se4[:, cs], AF.Ln)
                        nc.scalar.activation(rc4[:, cs], ln4[:, cs], AF.Exp, scale=-1.0)
                else:
                    nc.vector.tensor_copy(st0[:], ov0[:])
                    nc.scalar.copy(st1[:], ov1[:])
                    nc.sync.dma_start(se4[0:1, :], st0[64:65, :])
                    nc.scalar.dma_start(se4[1:2, :], st0[96:97, :])
                    nc.sync.dma_start(se4[2:3, :], st1[0:1, :])
                    nc.scalar.dma_start(se4[3:4, :], st1[32:33, :])
                    nc.scalar.activation(ln4[:], se4[:], AF.Ln)
                    nc.scalar.activation(rc4[:], ln4[:], AF.Exp, scale=-1.0)
                saved[grp] = (st0, st1, rc4)

            # fills[idx] runs after emit_scores of step idx. All psum-pool
            # fills MUST land before the first emit_pv touches the ov
            # accumulators (the ps_ot slots alias), so they are confined to
            # idx 0 and 1.
            fills = {
                0: [
                    lambda: qk_tile(0, chunks=(1,)),
                    lambda: qk_tile(2, chunks=(1,)),
                    lambda: v_tile(0),
                    lambda: v_tile(1),
                    lambda: v_tile(2),
                    lambda: v_tile(3),
                ],
                1: [
                    lambda: v_tile(4),
                    lambda: v_tile(5),
                    lambda: v_tile(6),
                    lambda: v_tile(7),
                    lambda: qk_tile(1),
                    lambda: qk_tile(3),
                ],
            }

            steps = [(grp, j, c) for grp in range(2) for j in range(8) for c in range(2)]
            pend = None
            for idx in range(len(steps) + 1):
                if idx < len(steps):
                    grp, j, c = steps[idx]
                    if (j, c) == (0, 0):
                        ovs[grp] = (
                            ps_ot.tile([128, 1024], f32, tag="ot", name=f"ov0_{grp}"),
                            ps_ot.tile([128, 1024], f32, tag="ot", name=f"ov1_{grp}"),
                        )
                    cur = (*emit_scores(grp, j, c), grp, j, c)
                    for fn in fills.get(idx, ()):
                        fn()
                else:
                    cur = None
                if pend is not None:
                    emit_pv(*pend)
                    if pend[3:5] == (7, 1):
                        emit_group_stage(pend[2])
                pend = cur

            # ---- deferred normalization + projection, pipelined by c-chunk ----
            # group 0's broadcast runs as soon as the last exp frees the s4a
            # slot; its scale-multiplies go to GpSimd (idle in the tail) so
            # the DVE only carries group 1's. Bias adds ride ScalarE (Copy
            # with bias). Each (m, c) output chunk DMAs out as soon as ready.
            bcs = {}
            for grp in range(2):
                bpool = ps_s4a if grp == 0 else ps_s4b
                bcs[grp] = bpool.tile(
                    [128, 1024], f32,
                    tag=("s4a" if grp == 0 else "s4b"), name=f"bc{grp}",
                )
            for c in range(2):
                cs = slice(512 * c, 512 * (c + 1))
                for grp in range(2):
                    st0, st1, rc4 = saved[grp]
                    bc = bcs[grp]
                    nc.tensor.matmul(
                        bc[:, cs], sel[:, :], rc4[:, cs], start=True, stop=True,
                    )
                    eng0 = nc.vector
                    eng1 = nc.vector
                    eng0.tensor_mul(
                        otf[0:64, 1024 * grp + 512 * c : 1024 * grp + 512 * (c + 1)],
                        st0[0:64, cs],
                        bc[0:64, cs],
                    )
                    eng1.tensor_mul(
                        otf[64:128, 1024 * grp + 512 * c : 1024 * grp + 512 * (c + 1)],
                        st1[64:128, cs],
                        bc[64:128, cs],
                    )
                for m in range(2):
                    ps = ps_ot.tile([128, 512], f32, tag="ot")
                    for t in range(2):
                        nc.tensor.matmul(
                            ps[:],
                            pw[:, 256 * t + 128 * m : 256 * t + 128 * (m + 1)],
                            otf[:, 1024 * t + 512 * c : 1024 * t + 512 * (c + 1)],
                            start=(t == 0),
                            stop=(t == 1),
                        )
                    ycs = slice(1024 * m + 512 * c, 1024 * m + 512 * (c + 1))
                    nc.vector.tensor_scalar_add(ytile[:, ycs], ps[:], pb[:, m : m + 1])
                    (nc.sync if m == 0 else nc.scalar).dma_start(
                        yt_d[:, ycs], ytile[:, ycs],
                    )

    return nc


def _get_nc():
    if "nc" not in _cached:
        _cached["nc"] = _build_nc()
    return _cached["nc"]


def _pack_per_partition(a2d):
    """[2*128, F] -> [128, 2*F] with tile t at cols F*t."""
    n, f = a2d.shape
    t = n // 128
    return np.ascontiguousarray(
        a2d.reshape(t, 128, f).transpose(1, 0, 2).reshape(128, t * f)
    )


def _prepare(carrier_tokens, ct_mask, batch_num_windows, qkv_w, qkv_b, proj_w, proj_b):
    """Host-side bookkeeping: ragged->padded scatter, weight packing.
    Returns (in_maps, ctx) where ctx carries what postprocessing needs."""
    import ml_dtypes

    bf16 = ml_dtypes.bfloat16
    carrier_tokens = np.asarray(carrier_tokens, dtype=np.float32)
    ct_mask = np.asarray(ct_mask, dtype=np.float32)
    lens = np.asarray(batch_num_windows).astype(np.int64)
    qkv_w = np.asarray(qkv_w, dtype=np.float32)
    qkv_b = np.asarray(qkv_b, dtype=np.float32)
    proj_w = np.asarray(proj_w, dtype=np.float32)
    proj_b = np.asarray(proj_b, dtype=np.float32)

    total = carrier_tokens.shape[0]

    # ragged -> padded bookkeeping (mirrors the reference's scatter semantics:
    # OOB scatter indices dropped, OOB gather indices clipped)
    offsets = np.concatenate([[0], np.cumsum(lens)])
    tok = np.arange(total)
    b_id = np.searchsorted(offsets[1:], tok, side="right")
    w_id = tok - offsets[np.minimum(b_id, B)]
    flat_idx = b_id * MAXW + w_id
    valid = flat_idx < B * MAXW
    padded = np.zeros((B * MAXW, C), np.float32)
    padded[flat_idx[valid]] = carrier_tokens[valid]
    padded = padded.reshape(B, MAXW, C)

    mask_col = np.ascontiguousarray(ct_mask[:, 0, :])  # [B, MAXW]

    # host-side exact weight transforms
    pw_perm = proj_w
    pb_eff = qkv_b[2 * C : 3 * C] @ proj_w + proj_b

    qw_packed = _pack_per_partition(qkv_w).astype(bf16)          # [128, 1536]
    qb_packed = np.ascontiguousarray(qkv_b[0:C].reshape(2, 128).T)
    pw_packed = _pack_per_partition(pw_perm).astype(bf16)        # [128, 512]
    pb_packed = np.ascontiguousarray(pb_eff.reshape(2, 128).T)

    onesb_arr = np.ones((128, 4), bf16)
    sel_arr = np.zeros((4, 128), np.float32)
    for k in range(4):
        sel_arr[k, 32 * k : 32 * (k + 1)] = 1.0
    sel_arr = sel_arr.astype(bf16)
    in_maps = []
    for b in range(B):
        xt = _pack_per_partition(padded[b].T).astype(bf16)       # [128, 2048]
        mb = np.ascontiguousarray(mask_col[b].reshape(8, 128).T)
        in_maps.append(
            {
                "xt": xt,
                "qw": qw_packed,
                "qb": qb_packed,
                "pw": pw_packed,
                "pb": pb_packed,
                "mask": mb,
                "onesb": onesb_arr,
                "sel": sel_arr,
            }
        )

    ctx = {
        "flat_idx": flat_idx,
        "mask_col": mask_col,
        "padded": padded,
        "qkv_w": qkv_w,
        "qkv_b": qkv_b,
        "proj_w": proj_w,
        "proj_b": proj_b,
    }
    return in_maps, ctx


def _postprocess(results, ctx):
    """Per-core outputs -> full ragged output (gather + degenerate-row fix)."""
    flat_idx = ctx["flat_idx"]
    mask_col = ctx["mask_col"]
    padded = ctx["padded"]
    qkv_w, qkv_b = ctx["qkv_w"], ctx["qkv_b"]
    proj_w, proj_b = ctx["proj_w"], ctx["proj_b"]

    y_pad = np.empty((B, MAXW, C), np.float32)
    for b in range(B):
        yt = results[b]["yt"]                                   # [128, 2048]
        y_t = yt.reshape(128, 2, MAXW).transpose(1, 0, 2).reshape(C, MAXW)
        y_pad[b] = y_t.T
    y_flat = y_pad.reshape(B * MAXW, C)
    gather_idx = np.clip(flat_idx, 0, B * MAXW - 1)
    out = y_flat[gather_idx]

    # degenerate rows: gathered positions whose key mask is fully masked.
    # The reference's softmax (with max-subtraction) gives uniform weights
    # there; our exp underflows to 0/0. Recompute those rows exactly.
    row_b = np.minimum(gather_idx // MAXW, B - 1)
    degenerate_batches = [b for b in range(B) if np.all(mask_col[b] < NEG_THRESH)]
    for b in degenerate_batches:
        rows = np.nonzero(row_b == b)[0]
        if rows.size == 0:
            continue
        vmat = padded[b] @ qkv_w[:, 2 * C : 3 * C] + qkv_b[2 * C : 3 * C]
        mean_v = vmat.mean(axis=0)  # uniform attention, same for all heads
        fix = mean_v @ proj_w + proj_b
        out[rows] = fix.astype(np.float32)

    return np.ascontiguousarray(out.astype(np.float32))


def run_device(in_maps, **spmd_kwargs):
    from concourse import bass_utils

    nc = _get_nc()
    return bass_utils.run_bass_kernel_spmd(
        nc, in_maps, core_ids=list(range(B)), **spmd_kwargs
    )


def kernel(carrier_tokens, ct_mask, batch_num_windows, qkv_w, qkv_b, proj_w, proj_b):
    in_maps, ctx = _prepare(
        carrier_tokens, ct_mask, batch_num_windows, qkv_w, qkv_b, proj_w, proj_b
    )
    res = run_device(in_maps, trace=False)
    return _postprocess(res.results, ctx)
